# revision 3
# baseline (speedup 1.0000x reference)
"""VQ codebook kernel for Trainium2 (Bass/Tile), 8-core data-parallel.

Reference math (per row n of x):
    dist[n,k] = sum_d alpha[d] * (x[n,d] - c[k,d])^2
    mapping   = softmax_k(dist)          # NOTE: softmax of +dist
    recon     = mapping @ C
    pred      = mapping @ sigmoid(w)

Since softmax is shift-invariant per row, the sum_d alpha*x^2 term cancels:
    logits[n,k] = t[k] - x[n,:] @ W2[:,k]
with W2[d,k] = 2*alpha[d]*c[k,d] and t[k] = sum_d alpha[d]*c[k,d]^2.

Device pipeline per 128-row tile (all f32):
    PE : transpose x tile (4x 128x128) -> xT in PSUM
    DVE: copy xT -> SBUF
    PE : g = xT.T @ W2 chunks, accumulated in PSUM       [128,5]
    DVE: lgneg = g - t, rowmin -> mn    (one fused tensor_tensor_reduce)
    ACT: e = Exp(-lgneg + mn), rowsum -> s  (one op, accum_out)
    DVE: r = 1/s
    PE : eT = transpose(e)  [5,128];  DVE copies eT -> SBUF
    PE : ur = eT.T @ C      [128,512] in PSUM (unnormalized recon)
    DVE: recon = ur * r     (normalize fused into PSUM->SBUF move)
    DVE: map = e * r ; pred = sum_k map*sigw (fused multiply-reduce)

Sharding: x rows split across 8 cores (8192 each), constants replicated.
"""

import numpy as np

import concourse.bacc as bacc
import concourse.bass as bass
import concourse.tile as tile
from concourse import mybir
from concourse.bass_utils import run_bass_kernel_spmd
from concourse.masks import make_identity

F32 = mybir.dt.float32
N, D, K = 65536, 512, 5
NCORES = 8
ROWS_PER_CORE = N // NCORES  # 8192
P = 128                      # partitions / rows per tile
DCH = D // P                 # 4 contraction chunks of 128
CHUNK_TILES = 16             # tiles per chunk
CHUNK_ROWS = P * CHUNK_TILES # 2048
STORE_GROUP = 4              # tiles per recon store DMA (1 MiB)


def build_program(rows_per_core=ROWS_PER_CORE, chunk_tiles=CHUNK_TILES):
    """Build + compile the single-core Tile program (same binary on all cores)."""
    chunk_rows = P * chunk_tiles
    assert rows_per_core % chunk_rows == 0
    n_chunks = rows_per_core // chunk_rows

    nc = bacc.Bacc("TRN2", target_bir_lowering=False, debug=False,
                   num_devices=NCORES)

    x_d = nc.dram_tensor("x", [rows_per_core, D], F32, kind="ExternalInput")
    w2_d = nc.dram_tensor("w2", [D, K], F32, kind="ExternalInput")
    t_d = nc.dram_tensor("tvec", [K], F32, kind="ExternalInput")
    sw_d = nc.dram_tensor("sigw", [K], F32, kind="ExternalInput")
    cats_d = nc.dram_tensor("cats", [K, D], F32, kind="ExternalInput")
    map_d = nc.dram_tensor("map_out", [rows_per_core, K], F32,
                           kind="ExternalOutput")
    rec_d = nc.dram_tensor("rec_out", [rows_per_core, D], F32,
                           kind="ExternalOutput")
    pred_d = nc.dram_tensor("pred_out", [rows_per_core], F32,
                            kind="ExternalOutput")

    # chunk/partition-major views: row index = c*chunk_rows + p*chunk_tiles + t
    # -> per-partition DMA segments are contiguous in DRAM.
    x_v = x_d.ap().rearrange("(c p t) d -> c p t d", p=P, t=chunk_tiles)
    map_v = map_d.ap().rearrange("(c p t) k -> c p t k", p=P, t=chunk_tiles)
    rec_v = rec_d.ap().rearrange("(c p t) d -> c p t d", p=P, t=chunk_tiles)
    pred_v = pred_d.ap().rearrange("(c p t) -> c p t", p=P, t=chunk_tiles)

    with tile.TileContext(nc) as tc:
        with (
            tc.tile_pool(name="consts", bufs=1) as consts,
            tc.tile_pool(name="xin", bufs=2) as xin_pool,
            tc.tile_pool(name="xt", bufs=3) as xt_pool,
            tc.tile_pool(name="small", bufs=3) as small,
            tc.tile_pool(name="outs", bufs=2) as outs_pool,
            tc.tile_pool(name="ps_xt", bufs=2, space="PSUM") as ps_xt,
            tc.tile_pool(name="ps_g", bufs=2, space="PSUM") as ps_g,
            tc.tile_pool(name="ps_et", bufs=2, space="PSUM") as ps_et,
            tc.tile_pool(name="ps_ur", bufs=2, space="PSUM") as ps_ur,
        ):
            identity = consts.tile([P, P], F32)
            make_identity(nc, identity)

            w2_s = consts.tile([P, DCH, K], F32)
            nc.gpsimd.dma_start(
                out=w2_s, in_=w2_d.ap().rearrange("(c p) k -> p c k", p=P))

            t_ap = t_d.ap()
            tb = consts.tile([P, K], F32)
            nc.gpsimd.dma_start(out=tb, in_=bass.AP(
                tensor=t_ap.tensor, offset=t_ap.offset,
                ap=[[0, P]] + list(t_ap.ap)))

            sw_ap = sw_d.ap()
            swb = consts.tile([P, K], F32)
            nc.gpsimd.dma_start(out=swb, in_=bass.AP(
                tensor=sw_ap.tensor, offset=sw_ap.offset,
                ap=[[0, P]] + list(sw_ap.ap)))

            cats_s = consts.tile([K, D], F32)
            nc.gpsimd.dma_start(out=cats_s, in_=cats_d.ap())

            for c in range(n_chunks):
                x_tile = xin_pool.tile([P, chunk_tiles, D], F32, tag="x")
                for j in range(0, chunk_tiles, STORE_GROUP):
                    nc.sync.dma_start(
                        out=x_tile[:, j:j + STORE_GROUP, :],
                        in_=x_v[c, :, j:j + STORE_GROUP, :])

                rec_t = outs_pool.tile([P, chunk_tiles, D], F32, tag="rec")
                map_t = outs_pool.tile([P, chunk_tiles, K], F32, tag="map")
                pred_t = outs_pool.tile([P, chunk_tiles], F32, tag="pred")

                for t in range(chunk_tiles):
                    # --- transpose x tile: 4x [128,128] -> PSUM, copy to SBUF
                    xt_ps = ps_xt.tile([P, D], F32, tag="xtp")
                    for j in range(DCH):
                        nc.tensor.transpose(
                            xt_ps[:, j * P:(j + 1) * P],
                            x_tile[:, t, j * P:(j + 1) * P],
                            identity)
                    xt_s = xt_pool.tile([P, D], F32, tag="xt")
                    nc.vector.tensor_copy(xt_s, xt_ps)

                    # --- distances: g[n,k] = sum_d xT[d,n]*W2[d,k]
                    g_ps = ps_g.tile([P, K], F32, tag="g")
                    for j in range(DCH):
                        nc.tensor.matmul(
                            g_ps, xt_s[:, j * P:(j + 1) * P], w2_s[:, j, :],
                            start=(j == 0), stop=(j == DCH - 1))

                    # --- lgneg = g - t ; mn = rowmin(lgneg)
                    lgneg = small.tile([P, K], F32, tag="lg")
                    mn = small.tile([P, 1], F32, tag="mn")
                    nc.vector.tensor_sub(lgneg, g_ps, tb)
                    nc.vector.tensor_reduce(
                        mn, lgneg, mybir.AxisListType.X, mybir.AluOpType.min)

                    # --- e = exp(mn - lgneg) = exp(logits - max); s = sum e
                    e_t = small.tile([P, K], F32, tag="e")
                    s_t = small.tile([P, 1], F32, tag="s")
                    nc.scalar.activation(
                        out=e_t, in_=lgneg,
                        func=mybir.ActivationFunctionType.Exp,
                        bias=mn, scale=-1.0, accum_out=s_t)

                    r_t = small.tile([P, 1], F32, tag="r")
                    nc.vector.reciprocal(r_t, s_t)

                    # --- mapping output + pred (fused multiply-reduce)
                    nc.vector.tensor_scalar_mul(map_t[:, t, :], e_t, r_t)
                    junk = small.tile([P, K], F32, tag="junk")
                    nc.vector.tensor_mul(junk, map_t[:, t, :], swb)
                    nc.vector.tensor_reduce(
                        pred_t[:, t:t + 1], junk, mybir.AxisListType.X,
                        mybir.AluOpType.add)

                    # --- recon: ur = eT.T @ C, then normalize by r on the
                    #     PSUM->SBUF move.
                    et_ps = ps_et.tile([K, P], F32, tag="et")
                    nc.tensor.transpose(et_ps, e_t, identity)
                    et_s = small.tile([K, P], F32, tag="ets")
                    nc.vector.tensor_copy(et_s, et_ps)

                    ur_ps = ps_ur.tile([P, D], F32, tag="ur")
                    nc.tensor.matmul(ur_ps, et_s, cats_s, start=True, stop=True)
                    nc.vector.tensor_scalar_mul(rec_t[:, t, :], ur_ps, r_t)

                    if t % STORE_GROUP == STORE_GROUP - 1:
                        j0 = t - (STORE_GROUP - 1)
                        nc.sync.dma_start(
                            out=rec_v[c, :, j0:t + 1, :],
                            in_=rec_t[:, j0:t + 1, :])

                nc.sync.dma_start(out=map_v[c], in_=map_t)
                nc.sync.dma_start(out=pred_v[c], in_=pred_t)

    nc.compile()
    return nc


_CACHE = {}


def _get_program():
    if "nc" not in _CACHE:
        _CACHE["nc"] = build_program()
    return _CACHE["nc"]


def make_host_constants(alpha_p, classif_w, centroids):
    alpha = np.asarray(alpha_p, dtype=np.float64)
    C = np.asarray(centroids, dtype=np.float64)
    w = np.asarray(classif_w, dtype=np.float64).reshape(-1)
    w2 = (2.0 * alpha[:, None] * C.T).astype(np.float32)        # [D,K]
    t = np.einsum("d,kd->k", alpha, C * C).astype(np.float32)   # [K]
    sigw = (1.0 / (1.0 + np.exp(-w))).astype(np.float32)        # [K]
    cats = np.ascontiguousarray(np.asarray(centroids, np.float32))  # [K,D]
    return w2, t, sigw, cats


def kernel(x, is_protected=None, alpha_p=None, classif_w=None, centroids=None,
           _trace=False, _trace_kwargs=None):
    x = np.ascontiguousarray(np.asarray(x, dtype=np.float32))
    assert x.shape == (N, D)
    w2, t, sigw, cats = make_host_constants(alpha_p, classif_w, centroids)

    nc = _get_program()
    in_maps = []
    for i in range(NCORES):
        shard = np.ascontiguousarray(
            x[i * ROWS_PER_CORE:(i + 1) * ROWS_PER_CORE])
        in_maps.append({"x": shard, "w2": w2, "tvec": t, "sigw": sigw,
                        "cats": cats})

    kwargs = {}
    if _trace:
        kwargs["trace"] = True
        if _trace_kwargs:
            kwargs.update(_trace_kwargs)
    res = run_bass_kernel_spmd(nc, in_maps, list(range(NCORES)), **kwargs)

    mapping = np.concatenate([r["map_out"] for r in res.results], axis=0)
    recon = np.concatenate([r["rec_out"] for r in res.results], axis=0)
    pred = np.concatenate([r["pred_out"] for r in res.results], axis=0)
    if _trace:
        _CACHE["last_results"] = res
    return (mapping.astype(np.float32, copy=False),
            recon.astype(np.float32, copy=False),
            pred.astype(np.float32, copy=False))
